# revision 11
# baseline (speedup 1.0000x reference)
"""Multi-head attention (B=2, S=2048, D=1024, H=16) on 8 trn2 NeuronCores.

Sharding: data-parallel over batch (2 groups of 4 cores), tensor-parallel over
heads within a group (4 heads/core).  Each core computes a partial output
(its heads' contribution through its W_o column shard); the host sums the 4
partials per batch element.

Schedule (v3): the kernel is ACT-bound (exp of 4x2048x2048 scores = ~142us at
1 elem/cycle/lane @1.2GHz), so everything else is scheduled around keeping the
exp stream dense:
  - inputs are pre-tiled on the host into s-512 chunks (contiguous 4-8KB DMA
    lines) and DMAed in consumption order: the first K/Q projections start
    ~7us in and the first exp fires ~15us in.
  - all remaining projection work (V-proj, later K/Q-proj chunks, output
    projection of finished chunks) is drip-fed as background PE work INSIDE
    the attention t-loops, filling the PE slack under each exp instruction.
  - scores run TWO k-tiles ahead of AV, so a stalled AV (e.g. waiting on an
    av-psum buffer still held by the previous block's epilogue) never stalls
    the exp stream.
  - each block's av psum is drained to SBUF immediately (releasing the psum
    bank for the next block) and the softmax-normalize epilogue is deferred
    into the NEXT block's background slots.
  - the denominator broadcast matmul runs in fp16 (1-pass, not 2-pass fp32),
    and a dummy exp hoists the ~2.7us ACT table load off the critical path.

Per-core layout (unchanged from v1):
  - q, k produced TRANSPOSED ([d_local, s]); scores lhsT = kT tile, rhs = qT,
    head pairs row-packed via tile_position (0,0)/(64,0).
  - v in natural layout with a fused ones-column so the AV matmul emits the
    attention output and the softmax denominator in one accumulation chain.
  - softmax skips max-subtraction (scores ~ N(0,1); fp32 exp cannot overflow).
"""

import os
from contextlib import ExitStack

import numpy as np

B = 2
S = 2048
DM = 1024
H = 16
DK = 64
P = 128
HC = 4            # heads per core
DO = HC * DK      # 256: local output dim of q/k/v projections
DI_T = DM // P    # 8 contraction tiles for projections
S_T = S // P      # 16
N_SC = 4          # s-chunks of 512

MM_BF16 = True    # matmul inputs in fp16 (fp32 PSUM accumulation everywhere)

_PROGRAM = None


def _build_program():
    import concourse.mybir as mybir
    import concourse.tile as tile
    from concourse import bacc

    f32 = mybir.dt.float32
    mmdt = mybir.dt.float16 if MM_BF16 else f32
    nc = bacc.Bacc("TRN2", target_bir_lowering=False, debug=False)

    # pre-tiled inputs: chunk sc is contiguous ([sc][p][t][512])
    qt_d = nc.dram_tensor("QTC", [N_SC, P, DI_T, 512], mmdt, kind="ExternalInput").ap()
    kt_d = nc.dram_tensor("KTC", [N_SC, P, DI_T, 512], mmdt, kind="ExternalInput").ap()
    vt_d = nc.dram_tensor("VTC", [N_SC, P, DI_T, 512], mmdt, kind="ExternalInput").ap()
    wqt_d = nc.dram_tensor("WQC", [P, DI_T, DO], mmdt, kind="ExternalInput").ap()
    wkt_d = nc.dram_tensor("WKC", [P, DI_T, DO], mmdt, kind="ExternalInput").ap()
    wvt_d = nc.dram_tensor("WVC", [P, DI_T, DO], mmdt, kind="ExternalInput").ap()
    wot_d = nc.dram_tensor("WOC", [P, 2, DM], mmdt, kind="ExternalInput").ap()
    out_d = nc.dram_tensor("OUT", [S, DM], mmdt, kind="ExternalOutput").ap()

    with tile.TileContext(nc) as tc, ExitStack() as ctx:
        _emit(ctx, tc, qt_d, kt_d, vt_d, wqt_d, wkt_d, wvt_d, wot_d, out_d)
    nc.compile()
    return nc


def _emit(ctx, tc, qt_d, kt_d, vt_d, wqt_d, wkt_d, wvt_d, wot_d, out_d):
    import concourse.mybir as mybir

    nc = tc.nc
    f32 = mybir.dt.float32
    mmdt = mybir.dt.float16 if MM_BF16 else f32
    Exp = mybir.ActivationFunctionType.Exp

    consts = ctx.enter_context(tc.tile_pool(name="consts", bufs=1))
    exp_pool = ctx.enter_context(tc.tile_pool(name="exp", bufs=8))
    smalls = ctx.enter_context(tc.tile_pool(name="smalls", bufs=2))
    avdrain = ctx.enter_context(tc.tile_pool(name="avdrain", bufs=2))
    ostage = ctx.enter_context(tc.tile_pool(name="ostage", bufs=2))

    qt_sb = consts.tile([P, DI_T, S], mmdt, tag="qt")     # staged Q^T
    kt_sb = consts.tile([P, DI_T, S], mmdt, tag="kt")
    vt_sb = consts.tile([P, DI_T, S], mmdt, tag="vt")
    wq_sb = consts.tile([P, DI_T, DO], mmdt, tag="wq")
    wk_sb = consts.tile([P, DI_T, DO], mmdt, tag="wk")
    wv_sb = consts.tile([P, DI_T, DO], mmdt, tag="wv")
    wo_sb = consts.tile([P, 2, DM], mmdt, tag="wo")
    qT_sb = consts.tile([P, 2, S], mmdt, tag="qT")    # [p, ot, s]; o_local = ot*128+p
    kT_sb = consts.tile([P, 2, S], mmdt, tag="kT")
    vab_sb = consts.tile([P, S_T, HC, DK + 1], mmdt, tag="vab")
    attnT_sb = consts.tile([P, 2, S], mmdt, tag="attnT")
    stage_sb = consts.tile([64, 2, S], mmdt, tag="oddstage")
    ones_sb = consts.tile([1, 64], mmdt, tag="ones")
    warm_sb = consts.tile([P, 512], mmdt, tag="warm")

    # chunked input DMAs; only the block-0 gating chunks are issued up front
    # so they get full DMA bandwidth — the rest are issued after the warmup.
    def in_chunk(dst, src, sc):
        nc.sync.dma_start(dst[:, :, sc * 512 : (sc + 1) * 512], src[sc])

    # sync queue (in-order): K and Q chunks, earliest-needed first
    in_chunk(kt_sb, kt_d, 0)
    in_chunk(qt_sb, qt_d, 0)
    in_chunk(kt_sb, kt_d, 1)
    in_chunk(kt_sb, kt_d, 2)
    in_chunk(kt_sb, kt_d, 3)
    in_chunk(qt_sb, qt_d, 1)
    in_chunk(qt_sb, qt_d, 2)
    in_chunk(qt_sb, qt_d, 3)

    # scalar queue (parallel channel): weights + V chunks; ACT is idle while
    # these issue, well before the first exp
    nc.scalar.dma_start(wk_sb[:], wkt_d)
    nc.scalar.dma_start(wq_sb[:], wqt_d)
    nc.scalar.dma_start(wv_sb[:], wvt_d)
    for sc in range(N_SC):
        nc.scalar.dma_start(vt_sb[:, :, sc * 512 : (sc + 1) * 512], vt_d[sc])
    nc.scalar.dma_start(wo_sb[:], wot_d)

    nc.vector.memset(vab_sb[:, :, :, DK : DK + 1], 1.0)
    nc.vector.memset(ones_sb[:], 1.0)
    nc.vector.memset(warm_sb[:], 0.0)

    psum_proj = ctx.enter_context(tc.tile_pool(name="psum_proj", bufs=2, space="PSUM"))
    psum_s_pool = ctx.enter_context(tc.tile_pool(name="psum_s", bufs=2, space="PSUM"))
    psum_av_pool = ctx.enter_context(tc.tile_pool(name="psum_av", bufs=2, space="PSUM"))

    # hoist the ~2.7us exp table load off the critical path (ACT is idle here)
    dummy_ex = smalls.tile([1, 8], f32, tag="dummyex")
    nc.scalar.activation(dummy_ex[:], warm_sb[0:1, 0:8], Exp)

    # PE warmup: flips the HAM clock gate to 8/8 and covers input-DMA latency
    for i in range(24):
        wp = psum_proj.tile([P, 512], f32, tag="proj", name="warmps")
        nc.tensor.matmul(wp[:], warm_sb[:, 0:P], warm_sb[:], start=True, stop=True)

    # ---- background PE work units -------------------------------------
    def kp_unit(sc, ot, src_sb=None, wsb=None, dst=None):
        # one 512-wide s-chunk, one o-tile of a transposed projection
        src_sb = kt_sb if src_sb is None else src_sb
        wsb = wk_sb if wsb is None else wsb
        dst = kT_sb if dst is None else dst
        pp = psum_proj.tile([P, 512], f32, tag="proj", name="pp")
        for t in range(DI_T):
            nc.tensor.matmul(
                pp[:],
                wsb[:, t, ot * P : (ot + 1) * P],
                src_sb[:, t, sc * 512 : (sc + 1) * 512],
                start=(t == 0),
                stop=(t == DI_T - 1),
            )
        nc.vector.tensor_copy(dst[:, ot, sc * 512 : (sc + 1) * 512], pp[:])

    def qp_unit(sc, ot):
        kp_unit(sc, ot, src_sb=qt_sb, wsb=wq_sb, dst=qT_sb)

    def vp_unit(st):
        # V projection s-tile (natural layout, into vab; ones column preset)
        pv = psum_proj.tile([P, 256], f32, tag="proj", name="pv")
        for t in range(DI_T):
            nc.tensor.matmul(
                pv[:],
                vt_sb[:, t, st * P : (st + 1) * P],
                wv_sb[:, t, :],
                start=(t == 0),
                stop=(t == DI_T - 1),
            )
        nc.vector.tensor_copy(
            vab_sb[:, st, :, 0:DK],
            pv[:].rearrange("p (h d) -> p h d", d=DK),
        )

    def op_unit(st):
        # output projection for one 128-row s-tile + its output DMA
        po = [
            psum_proj.tile([P, 512], f32, tag="proj", name=f"po{c}") for c in range(2)
        ]
        for col in range(2):
            for ot in range(2):
                nc.tensor.matmul(
                    po[col][:],
                    attnT_sb[:, ot, st * P : (st + 1) * P],
                    wo_sb[:, ot, col * 512 : (col + 1) * 512],
                    start=(ot == 0),
                    stop=(ot == 1),
                )
        ob = ostage.tile([P, DM], mmdt, tag="ostage")
        for col in range(2):
            nc.vector.tensor_copy(ob[:, col * 512 : (col + 1) * 512], po[col][:])
            nc.gpsimd.dma_start(
                out_d[st * P : (st + 1) * P, col * 512 : (col + 1) * 512],
                ob[:, col * 512 : (col + 1) * 512],
            )

    # ---- attention block ----------------------------------------------
    def block(ch, hp, bg, off=frozenset(), bg_first=False):
        """one (512-q-chunk, head-pair) attention block.

        bg: dict iteration -> list of background thunks, emitted into the
        PE stream at that t-iteration (fills PE slack under the exp stream).
        off: k-tiles whose exp runs on DVE (fp16 Schraudolph approximation,
        one tensor_scalar op) instead of ACT; their AV is delayed 2 extra
        iterations so a DVE backlog cannot stall the PE/ACT stream.
        Returns epilogue thunks (softmax normalize) to be run by the caller,
        normally deferred into the next block's bg slots.
        """
        q0 = ch * 512
        av = [
            psum_av_pool.tile([P, 512], f32, tag="av", name=f"av{j}") for j in range(2)
        ]
        ps_tiles = {}
        ex_tiles = {}

        def scores(t):
            ps_s = psum_s_pool.tile([P, 2, 512], f32, tag="scores", name="ps_s")
            ps_tiles[t] = ps_s
            for j in range(2):
                hb = j * 64
                nc.tensor.matmul(
                    ps_s[:, j, :],
                    kT_sb[hb : hb + 64, hp, t * P : (t + 1) * P],
                    qT_sb[hb : hb + 64, hp, q0 : q0 + 512],
                    start=True,
                    stop=True,
                    tile_position=(hb, 0),
                )

        def expop(t):
            ex = exp_pool.tile([P, 2, 512], mmdt, tag="exp", name="ex")
            ex_tiles[t] = ex
            if t in off:
                # fp16 Schraudolph on DVE: i16 = rne(s*(0.125*2^10/ln2) + b);
                # the int16 bit pattern IS the fp16 approximation of exp(s/8)
                nc.vector.tensor_scalar(
                    ex[:].bitcast(mybir.dt.int16),
                    ps_tiles.pop(t)[:],
                    184.6649475,
                    15264.0,
                    mybir.AluOpType.mult,
                    mybir.AluOpType.add,
                )
            else:
                nc.scalar.activation(ex[:], ps_tiles.pop(t)[:], Exp, scale=0.125)

        emitted = set()

        def avop(t):
            emitted.add(t)
            ex = ex_tiles.pop(t)
            for j in range(2):
                nc.tensor.matmul(
                    av[j][0 : DK + 1, :],
                    vab_sb[:, t, 2 * hp + j, :],
                    ex[:, j, :],
                    start=(t == 0),
                    stop=(t == S_T - 1),
                )

        # scores run 2 k-tiles ahead of AV (4 for DVE-offloaded tiles) so AV
        # stalls can't starve ACT
        scores(0)
        scores(1)
        expop(0)
        for t in range(2, S_T):
            due = []
            if t - 2 >= 0 and t - 2 not in off:
                due.append(t - 2)
            if t - 4 >= 0 and t - 4 in off:
                due.append(t - 4)
            if bg_first:
                for fn in bg.get(t, ()):
                    fn()
                for u in due:
                    avop(u)
            else:
                for u in due:
                    avop(u)
                for fn in bg.get(t, ()):
                    fn()
            scores(t)
            expop(t - 1)
        if S_T - 2 not in off:
            avop(S_T - 2)
        expop(S_T - 1)
        for t in sorted(set(range(S_T - 1)) - emitted):
            avop(t)
        avop(S_T - 1)

        # drain av psum to SBUF immediately: releases the av banks for the
        # next block; the normalize epilogue reads the SBUF copy later.
        avs = []
        for j in range(2):
            a = avdrain.tile([DK + 1, 512], f32, tag=f"avs{j}", name=f"avs{j}")
            nc.vector.tensor_copy(a[:], av[j][0 : DK + 1, :])
            avs.append(a)

        def epi(j):
            # softmax normalize for head j: fp16 1-pass denominator broadcast
            den_row = smalls.tile([1, 512], mmdt, tag="den", name=f"den{j}")
            nc.vector.tensor_copy(den_row[:], avs[j][DK : DK + 1, :])
            den_b = psum_s_pool.tile([64, 512], f32, tag="scores", name="den_b")
            nc.tensor.matmul(den_b[:], ones_sb[:], den_row[:], start=True, stop=True)
            rec_b = smalls.tile([64, 512], f32, tag="recb", name=f"rec{j}")
            nc.vector.reciprocal_approx_fast(rec_b[:], den_b[:])
            lh = 2 * hp + j
            if lh % 2 == 0:
                nc.vector.tensor_mul(
                    attnT_sb[0:64, lh // 2, q0 : q0 + 512], avs[j][0:DK, :], rec_b[:]
                )
            else:
                nc.vector.tensor_mul(
                    stage_sb[:, lh // 2, q0 : q0 + 512], avs[j][0:DK, :], rec_b[:]
                )
                nc.sync.dma_start(
                    attnT_sb[64:128, lh // 2, q0 : q0 + 512],
                    stage_sb[:, lh // 2, q0 : q0 + 512],
                )

        return [lambda: epi(0), lambda: epi(1)]

    # ---- schedule ------------------------------------------------------
    # prologue: minimum needed for block(0,0) t=0
    kp_unit(0, 0)
    kp_unit(0, 1)
    qp_unit(0, 0)
    qp_unit(0, 1)

    # block(0,0): pipeline fill — remaining K-proj chunks before their scores
    # tiles, V-proj tiles just-in-time before their AV tiles.
    epi = block(0, 0, {
        2: [lambda: vp_unit(0), lambda: kp_unit(1, 0)],
        3: [lambda: vp_unit(1), lambda: vp_unit(2), lambda: kp_unit(1, 1)],
        4: [lambda: vp_unit(3), lambda: vp_unit(4)],
        5: [lambda: kp_unit(2, 0), lambda: vp_unit(5)],
        6: [lambda: kp_unit(2, 1), lambda: vp_unit(6)],
        7: [lambda: vp_unit(7), lambda: vp_unit(8)],
        8: [lambda: kp_unit(3, 0), lambda: vp_unit(9)],
        9: [lambda: kp_unit(3, 1), lambda: vp_unit(10)],
        10: [lambda: vp_unit(11), lambda: vp_unit(12)],
        11: [lambda: vp_unit(13)],
        12: [lambda: vp_unit(14)],
        13: [lambda: vp_unit(15)],
    }, bg_first=True)
    OFF = frozenset({3, 7, 11, 13})
    epi = block(0, 1, {
        2: [epi[0]],
        3: [epi[1]],
        5: [lambda: qp_unit(1, 0)],
        9: [lambda: qp_unit(1, 1)],
    }, off=OFF)
    epi = block(1, 0, {
        2: [epi[0]],
        3: [epi[1]],
        5: [lambda: op_unit(0)],
        8: [lambda: op_unit(1)],
        11: [lambda: op_unit(2)],
        13: [lambda: op_unit(3)],
    }, off=OFF)
    epi = block(1, 1, {
        2: [epi[0]],
        3: [epi[1]],
        5: [lambda: qp_unit(2, 0)],
        9: [lambda: qp_unit(2, 1)],
    }, off=OFF)
    epi = block(2, 0, {
        2: [epi[0]],
        3: [epi[1]],
        5: [lambda: op_unit(4)],
        8: [lambda: op_unit(5)],
        11: [lambda: op_unit(6)],
        13: [lambda: op_unit(7)],
    }, off=OFF)
    epi = block(2, 1, {
        2: [epi[0]],
        3: [epi[1]],
        5: [lambda: qp_unit(3, 0)],
        9: [lambda: qp_unit(3, 1)],
    }, off=OFF)
    epi = block(3, 0, {
        2: [epi[0]],
        3: [epi[1]],
        5: [lambda: op_unit(8)],
        8: [lambda: op_unit(9)],
        11: [lambda: op_unit(10)],
        13: [lambda: op_unit(11)],
    }, off=OFF)
    epi = block(3, 1, {
        2: [epi[0]],
        3: [epi[1]],
    }, off=OFF)
    # keep the PE clock hot through the serial epilogue so the final output
    # projections run at full rate
    for i in range(8):
        wp = psum_proj.tile([P, 512], f32, tag="proj", name="tailwarm")
        nc.tensor.matmul(wp[:], warm_sb[:, 0:P], warm_sb[:], start=True, stop=True)
    epi[0]()
    epi[1]()
    for st in range(12, 16):
        op_unit(st)


def _get_program():
    global _PROGRAM
    if _PROGRAM is None:
        _PROGRAM = _build_program()
    return _PROGRAM


def make_in_maps(Q, K, V, W_q, W_k, W_v, W_o):
    """Per-core input dicts: core c -> batch c//4, heads (c%4)*4 ... +4.

    Inputs are pre-tiled so each DMA chunk is contiguous:
      KTC[sc, p, t, s'] = K^T[t*128+p, sc*512+s']   (likewise QTC/VTC)
      WKC[p, t, o]      = W_k^T[t*128+p, o]          (likewise WQC/WVC)
      WOC[p, ot, o]     = W_o^T[ot*128+p, o]
    """
    mmdt = np.float16 if MM_BF16 else np.float32

    def tile_in(x):  # [S, DM] -> x.T pre-tiled [4, 128, 8, 512]
        return np.ascontiguousarray(
            x.T.reshape(DI_T, P, N_SC, 512).transpose(2, 1, 0, 3)
        ).astype(mmdt)

    def tile_w(w):  # [DO, DM] -> w.T pre-tiled [128, 8, 256]
        return np.ascontiguousarray(
            w.T.reshape(DI_T, P, DO).transpose(1, 0, 2)
        ).astype(mmdt)

    in_maps = []
    for c in range(8):
        b, g = c // 4, c % 4
        sl = slice(g * DO, (g + 1) * DO)
        in_maps.append(
            {
                "QTC": tile_in(Q[b]),
                "KTC": tile_in(K[b]),
                "VTC": tile_in(V[b]),
                "WQC": tile_w(W_q[sl, :]),
                "WKC": tile_w(W_k[sl, :]),
                "WVC": tile_w(W_v[sl, :]),
                "WOC": np.ascontiguousarray(
                    W_o[:, sl].T.reshape(2, P, DM).transpose(1, 0, 2)
                ).astype(mmdt),
            }
        )
    return in_maps


def combine_outputs(outs):
    """outs: list of 8 [S, DM] partials -> [B, S, DM]."""
    o = [np.asarray(x, dtype=np.float32) for x in outs]
    return np.stack([o[0] + o[1] + o[2] + o[3], o[4] + o[5] + o[6] + o[7]])


def kernel(Q, K, V, W_q, W_k, W_v, W_o):
    from concourse.bass_utils import run_bass_kernel_spmd

    Q = np.asarray(Q)
    K = np.asarray(K)
    V = np.asarray(V)
    nc = _get_program()
    in_maps = make_in_maps(Q, K, V, np.asarray(W_q), np.asarray(W_k), np.asarray(W_v), np.asarray(W_o))
    res = run_bass_kernel_spmd(nc, in_maps, core_ids=list(range(8)))
    return combine_outputs([res.results[c]["OUT"] for c in range(8)])


# revision 12
# speedup vs baseline: 1.0696x; 1.0696x over previous
"""Multi-head attention (B=2, S=2048, D=1024, H=16) on 8 trn2 NeuronCores.

Sharding: data-parallel over batch (2 groups of 4 cores), tensor-parallel over
heads within a group (4 heads/core).  Each core computes a partial output
(its heads' contribution through its W_o column shard); the host sums the 4
partials per batch element.

Schedule (v3): the kernel is ACT-bound (exp of 4x2048x2048 scores = ~142us at
1 elem/cycle/lane @1.2GHz), so everything else is scheduled around keeping the
exp stream dense:
  - inputs are pre-tiled on the host into s-512 chunks (contiguous 4-8KB DMA
    lines) and DMAed in consumption order: the first K/Q projections start
    ~7us in and the first exp fires ~15us in.
  - all remaining projection work (V-proj, later K/Q-proj chunks, output
    projection of finished chunks) is drip-fed as background PE work INSIDE
    the attention t-loops, filling the PE slack under each exp instruction.
  - scores run TWO k-tiles ahead of AV, so a stalled AV (e.g. waiting on an
    av-psum buffer still held by the previous block's epilogue) never stalls
    the exp stream.
  - each block's av psum is drained to SBUF immediately (releasing the psum
    bank for the next block) and the softmax-normalize epilogue is deferred
    into the NEXT block's background slots.
  - the denominator broadcast matmul runs in fp16 (1-pass, not 2-pass fp32),
    and a dummy exp hoists the ~2.7us ACT table load off the critical path.

Per-core layout (unchanged from v1):
  - q, k produced TRANSPOSED ([d_local, s]); scores lhsT = kT tile, rhs = qT,
    head pairs row-packed via tile_position (0,0)/(64,0).
  - v in natural layout with a fused ones-column so the AV matmul emits the
    attention output and the softmax denominator in one accumulation chain.
  - softmax skips max-subtraction (scores ~ N(0,1); fp32 exp cannot overflow).
"""

import os
from contextlib import ExitStack

import numpy as np

B = 2
S = 2048
DM = 1024
H = 16
DK = 64
P = 128
HC = 4            # heads per core
DO = HC * DK      # 256: local output dim of q/k/v projections
DI_T = DM // P    # 8 contraction tiles for projections
S_T = S // P      # 16
N_SC = 4          # s-chunks of 512

MM_BF16 = True    # matmul inputs in fp16 (fp32 PSUM accumulation everywhere)

_PROGRAM = None


def _build_program():
    import concourse.mybir as mybir
    import concourse.tile as tile
    from concourse import bacc

    f32 = mybir.dt.float32
    mmdt = mybir.dt.float16 if MM_BF16 else f32
    nc = bacc.Bacc("TRN2", target_bir_lowering=False, debug=False)

    # pre-tiled inputs: chunk sc is contiguous ([sc][p][t][512])
    qt_d = nc.dram_tensor("QTC", [N_SC, P, DI_T, 512], mmdt, kind="ExternalInput").ap()
    kt_d = nc.dram_tensor("KTC", [N_SC, P, DI_T, 512], mmdt, kind="ExternalInput").ap()
    vt_d = nc.dram_tensor("VTC", [N_SC, P, DI_T, 512], mmdt, kind="ExternalInput").ap()
    wqt_d = nc.dram_tensor("WQC", [P, DI_T, DO], mmdt, kind="ExternalInput").ap()
    wkt_d = nc.dram_tensor("WKC", [P, DI_T, DO], mmdt, kind="ExternalInput").ap()
    wvt_d = nc.dram_tensor("WVC", [P, DI_T, DO], mmdt, kind="ExternalInput").ap()
    wot_d = nc.dram_tensor("WOC", [P, 2, DM], mmdt, kind="ExternalInput").ap()
    out_d = nc.dram_tensor("OUT", [S, DM], mmdt, kind="ExternalOutput").ap()

    with tile.TileContext(nc) as tc, ExitStack() as ctx:
        _emit(ctx, tc, qt_d, kt_d, vt_d, wqt_d, wkt_d, wvt_d, wot_d, out_d)
    nc.compile()
    return nc


def _emit(ctx, tc, qt_d, kt_d, vt_d, wqt_d, wkt_d, wvt_d, wot_d, out_d):
    import concourse.mybir as mybir

    nc = tc.nc
    f32 = mybir.dt.float32
    mmdt = mybir.dt.float16 if MM_BF16 else f32
    Exp = mybir.ActivationFunctionType.Exp

    consts = ctx.enter_context(tc.tile_pool(name="consts", bufs=1))
    exp_pool = ctx.enter_context(tc.tile_pool(name="exp", bufs=3))
    smalls = ctx.enter_context(tc.tile_pool(name="smalls", bufs=2))
    avdrain = ctx.enter_context(tc.tile_pool(name="avdrain", bufs=2))
    ostage = ctx.enter_context(tc.tile_pool(name="ostage", bufs=2))

    qt_sb = consts.tile([P, DI_T, S], mmdt, tag="qt")     # staged Q^T
    kt_sb = consts.tile([P, DI_T, S], mmdt, tag="kt")
    vt_sb = consts.tile([P, DI_T, S], mmdt, tag="vt")
    wq_sb = consts.tile([P, DI_T, DO], mmdt, tag="wq")
    wk_sb = consts.tile([P, DI_T, DO], mmdt, tag="wk")
    wv_sb = consts.tile([P, DI_T, DO], mmdt, tag="wv")
    wo_sb = consts.tile([P, 2, DM], mmdt, tag="wo")
    qT_sb = consts.tile([P, 2, S], mmdt, tag="qT")    # [p, ot, s]; o_local = ot*128+p
    kT_sb = consts.tile([P, 2, S], mmdt, tag="kT")
    vab_sb = consts.tile([P, S_T, HC, DK + 1], mmdt, tag="vab")
    attnT_sb = consts.tile([P, 2, S], mmdt, tag="attnT")
    stage_sb = consts.tile([64, 2, S], mmdt, tag="oddstage")
    ones_sb = consts.tile([1, 64], mmdt, tag="ones")
    warm_sb = consts.tile([P, 512], mmdt, tag="warm")

    # chunked input DMAs in consumption-priority order; kt0/qt0 gate block 0
    def in_chunk(dst, src, sc):
        nc.sync.dma_start(dst[:, :, sc * 512 : (sc + 1) * 512], src[sc])

    nc.sync.dma_start(wk_sb[:], wkt_d)
    nc.sync.dma_start(wq_sb[:], wqt_d)
    in_chunk(kt_sb, kt_d, 0)
    in_chunk(qt_sb, qt_d, 0)
    nc.sync.dma_start(wv_sb[:], wvt_d)
    in_chunk(vt_sb, vt_d, 0)
    in_chunk(kt_sb, kt_d, 1)
    in_chunk(kt_sb, kt_d, 2)
    in_chunk(kt_sb, kt_d, 3)
    in_chunk(vt_sb, vt_d, 1)
    in_chunk(vt_sb, vt_d, 2)
    in_chunk(vt_sb, vt_d, 3)
    in_chunk(qt_sb, qt_d, 1)
    in_chunk(qt_sb, qt_d, 2)
    in_chunk(qt_sb, qt_d, 3)
    nc.sync.dma_start(wo_sb[:], wot_d)

    nc.vector.memset(vab_sb[:, :, :, DK : DK + 1], 1.0)
    nc.vector.memset(ones_sb[:], 1.0)
    nc.vector.memset(warm_sb[:], 0.0)

    psum_proj = ctx.enter_context(tc.tile_pool(name="psum_proj", bufs=2, space="PSUM"))
    psum_s_pool = ctx.enter_context(tc.tile_pool(name="psum_s", bufs=2, space="PSUM"))
    psum_av_pool = ctx.enter_context(tc.tile_pool(name="psum_av", bufs=2, space="PSUM"))

    # hoist the ~2.7us exp table load off the critical path (ACT is idle here)
    dummy_ex = smalls.tile([1, 8], f32, tag="dummyex")
    nc.scalar.activation(dummy_ex[:], warm_sb[0:1, 0:8], Exp)

    # PE warmup: flips the HAM clock gate to 8/8 and covers input-DMA latency
    for i in range(24):
        wp = psum_proj.tile([P, 512], f32, tag="proj", name="warmps")
        nc.tensor.matmul(wp[:], warm_sb[:, 0:P], warm_sb[:], start=True, stop=True)

    # ---- background PE work units -------------------------------------
    def kp_unit(sc, ot, src_sb=None, wsb=None, dst=None):
        # one 512-wide s-chunk, one o-tile of a transposed projection
        src_sb = kt_sb if src_sb is None else src_sb
        wsb = wk_sb if wsb is None else wsb
        dst = kT_sb if dst is None else dst
        pp = psum_proj.tile([P, 512], f32, tag="proj", name="pp")
        for t in range(DI_T):
            nc.tensor.matmul(
                pp[:],
                wsb[:, t, ot * P : (ot + 1) * P],
                src_sb[:, t, sc * 512 : (sc + 1) * 512],
                start=(t == 0),
                stop=(t == DI_T - 1),
            )
        nc.vector.tensor_copy(dst[:, ot, sc * 512 : (sc + 1) * 512], pp[:])

    def qp_unit(sc, ot):
        kp_unit(sc, ot, src_sb=qt_sb, wsb=wq_sb, dst=qT_sb)

    def vp_unit(st):
        # V projection s-tile (natural layout, into vab; ones column preset)
        pv = psum_proj.tile([P, 256], f32, tag="proj", name="pv")
        for t in range(DI_T):
            nc.tensor.matmul(
                pv[:],
                vt_sb[:, t, st * P : (st + 1) * P],
                wv_sb[:, t, :],
                start=(t == 0),
                stop=(t == DI_T - 1),
            )
        nc.vector.tensor_copy(
            vab_sb[:, st, :, 0:DK],
            pv[:].rearrange("p (h d) -> p h d", d=DK),
        )

    def op_unit(st):
        # output projection for one 128-row s-tile + its output DMA
        po = [
            psum_proj.tile([P, 512], f32, tag="proj", name=f"po{c}") for c in range(2)
        ]
        for col in range(2):
            for ot in range(2):
                nc.tensor.matmul(
                    po[col][:],
                    attnT_sb[:, ot, st * P : (st + 1) * P],
                    wo_sb[:, ot, col * 512 : (col + 1) * 512],
                    start=(ot == 0),
                    stop=(ot == 1),
                )
        ob = ostage.tile([P, DM], mmdt, tag="ostage")
        for col in range(2):
            nc.vector.tensor_copy(ob[:, col * 512 : (col + 1) * 512], po[col][:])
            nc.gpsimd.dma_start(
                out_d[st * P : (st + 1) * P, col * 512 : (col + 1) * 512],
                ob[:, col * 512 : (col + 1) * 512],
            )

    # ---- attention block ----------------------------------------------
    def block(ch, hp, bg):
        """one (512-q-chunk, head-pair) attention block.

        bg: dict iteration -> list of background thunks, emitted into the
        PE stream at that t-iteration (fills PE slack under the exp stream).
        Returns epilogue thunks (softmax normalize) to be run by the caller,
        normally deferred into the next block's bg slots.
        """
        q0 = ch * 512
        av = [
            psum_av_pool.tile([P, 512], f32, tag="av", name=f"av{j}") for j in range(2)
        ]
        ps_tiles = {}
        ex_tiles = {}

        def scores(t):
            ps_s = psum_s_pool.tile([P, 2, 512], f32, tag="scores", name="ps_s")
            ps_tiles[t] = ps_s
            for j in range(2):
                hb = j * 64
                nc.tensor.matmul(
                    ps_s[:, j, :],
                    kT_sb[hb : hb + 64, hp, t * P : (t + 1) * P],
                    qT_sb[hb : hb + 64, hp, q0 : q0 + 512],
                    start=True,
                    stop=True,
                    tile_position=(hb, 0),
                )

        def expop(t):
            ex = exp_pool.tile([P, 2, 512], mmdt, tag="exp", name="ex")
            ex_tiles[t] = ex
            nc.scalar.activation(ex[:], ps_tiles.pop(t)[:], Exp, scale=0.125)

        def avop(t):
            ex = ex_tiles.pop(t)
            for j in range(2):
                nc.tensor.matmul(
                    av[j][0 : DK + 1, :],
                    vab_sb[:, t, 2 * hp + j, :],
                    ex[:, j, :],
                    start=(t == 0),
                    stop=(t == S_T - 1),
                )

        # scores run 2 k-tiles ahead of AV so AV stalls can't starve ACT
        scores(0)
        scores(1)
        expop(0)
        for t in range(2, S_T):
            avop(t - 2)
            for fn in bg.get(t, ()):
                fn()
            scores(t)
            expop(t - 1)
        avop(S_T - 2)
        expop(S_T - 1)
        avop(S_T - 1)

        # drain av psum to SBUF immediately: releases the av banks for the
        # next block; the normalize epilogue reads the SBUF copy later.
        avs = []
        for j in range(2):
            a = avdrain.tile([DK + 1, 512], f32, tag=f"avs{j}", name=f"avs{j}")
            nc.vector.tensor_copy(a[:], av[j][0 : DK + 1, :])
            avs.append(a)

        def epi(j):
            # softmax normalize for head j: fp16 1-pass denominator broadcast
            den_row = smalls.tile([1, 512], mmdt, tag="den", name=f"den{j}")
            nc.vector.tensor_copy(den_row[:], avs[j][DK : DK + 1, :])
            den_b = psum_s_pool.tile([64, 512], f32, tag="scores", name="den_b")
            nc.tensor.matmul(den_b[:], ones_sb[:], den_row[:], start=True, stop=True)
            rec_b = smalls.tile([64, 512], f32, tag="recb", name=f"rec{j}")
            nc.vector.reciprocal_approx_fast(rec_b[:], den_b[:])
            lh = 2 * hp + j
            if lh % 2 == 0:
                nc.vector.tensor_mul(
                    attnT_sb[0:64, lh // 2, q0 : q0 + 512], avs[j][0:DK, :], rec_b[:]
                )
            else:
                nc.vector.tensor_mul(
                    stage_sb[:, lh // 2, q0 : q0 + 512], avs[j][0:DK, :], rec_b[:]
                )
                nc.sync.dma_start(
                    attnT_sb[64:128, lh // 2, q0 : q0 + 512],
                    stage_sb[:, lh // 2, q0 : q0 + 512],
                )

        return [lambda: epi(0), lambda: epi(1)]

    # ---- schedule ------------------------------------------------------
    # prologue: minimum needed for block(0,0) t=0
    kp_unit(0, 0)
    kp_unit(0, 1)
    qp_unit(0, 0)
    qp_unit(0, 1)
    vp_unit(0)

    # block(0,0): pipeline fill — remaining K-proj chunks before their scores
    # tiles, V-proj tiles just-in-time before their AV tiles.
    epi = block(0, 0, {
        2: [lambda: vp_unit(1), lambda: kp_unit(1, 0)],
        3: [lambda: vp_unit(2), lambda: kp_unit(1, 1)],
        4: [lambda: vp_unit(3), lambda: vp_unit(4)],
        5: [lambda: kp_unit(2, 0), lambda: vp_unit(5)],
        6: [lambda: kp_unit(2, 1), lambda: vp_unit(6)],
        7: [lambda: vp_unit(7), lambda: vp_unit(8)],
        8: [lambda: kp_unit(3, 0), lambda: vp_unit(9)],
        9: [lambda: kp_unit(3, 1), lambda: vp_unit(10)],
        10: [lambda: vp_unit(11), lambda: vp_unit(12)],
        11: [lambda: vp_unit(13)],
        12: [lambda: vp_unit(14)],
        13: [lambda: vp_unit(15)],
    })
    epi = block(0, 1, {
        2: [epi[0]],
        3: [epi[1]],
        5: [lambda: qp_unit(1, 0)],
        9: [lambda: qp_unit(1, 1)],
    })
    epi = block(1, 0, {
        2: [epi[0]],
        3: [epi[1]],
        5: [lambda: op_unit(0)],
        8: [lambda: op_unit(1)],
        11: [lambda: op_unit(2)],
        13: [lambda: op_unit(3)],
    })
    epi = block(1, 1, {
        2: [epi[0]],
        3: [epi[1]],
        5: [lambda: qp_unit(2, 0)],
        9: [lambda: qp_unit(2, 1)],
    })
    epi = block(2, 0, {
        2: [epi[0]],
        3: [epi[1]],
        5: [lambda: op_unit(4)],
        8: [lambda: op_unit(5)],
        11: [lambda: op_unit(6)],
        13: [lambda: op_unit(7)],
    })
    epi = block(2, 1, {
        2: [epi[0]],
        3: [epi[1]],
        5: [lambda: qp_unit(3, 0)],
        9: [lambda: qp_unit(3, 1)],
    })
    epi = block(3, 0, {
        2: [epi[0]],
        3: [epi[1]],
        5: [lambda: op_unit(8)],
        8: [lambda: op_unit(9)],
        11: [lambda: op_unit(10)],
        13: [lambda: op_unit(11)],
    })
    epi = block(3, 1, {
        2: [epi[0]],
        3: [epi[1]],
    })
    # keep the PE clock hot through the serial epilogue so the final output
    # projections run at full rate
    for i in range(8):
        wp = psum_proj.tile([P, 512], f32, tag="proj", name="tailwarm")
        nc.tensor.matmul(wp[:], warm_sb[:, 0:P], warm_sb[:], start=True, stop=True)
    epi[0]()
    epi[1]()
    for st in range(12, 16):
        op_unit(st)


def _get_program():
    global _PROGRAM
    if _PROGRAM is None:
        _PROGRAM = _build_program()
    return _PROGRAM


def make_in_maps(Q, K, V, W_q, W_k, W_v, W_o):
    """Per-core input dicts: core c -> batch c//4, heads (c%4)*4 ... +4.

    Inputs are pre-tiled so each DMA chunk is contiguous:
      KTC[sc, p, t, s'] = K^T[t*128+p, sc*512+s']   (likewise QTC/VTC)
      WKC[p, t, o]      = W_k^T[t*128+p, o]          (likewise WQC/WVC)
      WOC[p, ot, o]     = W_o^T[ot*128+p, o]
    """
    mmdt = np.float16 if MM_BF16 else np.float32

    def tile_in(x):  # [S, DM] -> x.T pre-tiled [4, 128, 8, 512]
        return np.ascontiguousarray(
            x.T.reshape(DI_T, P, N_SC, 512).transpose(2, 1, 0, 3)
        ).astype(mmdt)

    def tile_w(w):  # [DO, DM] -> w.T pre-tiled [128, 8, 256]
        return np.ascontiguousarray(
            w.T.reshape(DI_T, P, DO).transpose(1, 0, 2)
        ).astype(mmdt)

    in_maps = []
    for c in range(8):
        b, g = c // 4, c % 4
        sl = slice(g * DO, (g + 1) * DO)
        in_maps.append(
            {
                "QTC": tile_in(Q[b]),
                "KTC": tile_in(K[b]),
                "VTC": tile_in(V[b]),
                "WQC": tile_w(W_q[sl, :]),
                "WKC": tile_w(W_k[sl, :]),
                "WVC": tile_w(W_v[sl, :]),
                "WOC": np.ascontiguousarray(
                    W_o[:, sl].T.reshape(2, P, DM).transpose(1, 0, 2)
                ).astype(mmdt),
            }
        )
    return in_maps


def combine_outputs(outs):
    """outs: list of 8 [S, DM] partials -> [B, S, DM]."""
    o = [np.asarray(x, dtype=np.float32) for x in outs]
    return np.stack([o[0] + o[1] + o[2] + o[3], o[4] + o[5] + o[6] + o[7]])


def kernel(Q, K, V, W_q, W_k, W_v, W_o):
    from concourse.bass_utils import run_bass_kernel_spmd

    Q = np.asarray(Q)
    K = np.asarray(K)
    V = np.asarray(V)
    nc = _get_program()
    in_maps = make_in_maps(Q, K, V, np.asarray(W_q), np.asarray(W_k), np.asarray(W_v), np.asarray(W_o))
    res = run_bass_kernel_spmd(nc, in_maps, core_ids=list(range(8)))
    return combine_outputs([res.results[c]["OUT"] for c in range(8)])
